# revision 1
# baseline (speedup 1.0000x reference)
import math
import numpy as np
import jax
import jax.numpy as jnp
from jax.sharding import Mesh, PartitionSpec as P
from jax.experimental.shard_map import shard_map

# Model dims (hardcoded per spec)
B, T, N, C = 16, 24, 207, 64
E = 16
L = 3
NH = 4
HD = C // NH
HOR = 12
NDEV = 8
BL = B // NDEV  # batch per core

PARAM_NAMES = [
    'static_adj', 'node_embedding',
    'hour_w', 'hour_b', 'day_w', 'day_b', 'weekend_w', 'weekend_b',
    'in_w', 'in_b',
    'sa_qw', 'sa_qb', 'sa_kw', 'sa_kb', 'sa_vw', 'sa_vb',
    'ta_qw', 'ta_qb', 'ta_kw', 'ta_kb', 'ta_vw', 'ta_vb', 'ta_ow', 'ta_ob',
    'ln1_w', 'ln1_b', 'ln2_w', 'ln2_b',
    'ffn_w1', 'ffn_b1', 'ffn_w2', 'ffn_b2',
    'gru_wih', 'gru_whh', 'gru_bih', 'gru_bhh',
    'pred_w', 'pred_b',
]


def _ln(x, w, b, eps=1e-5):
    m = x.mean(-1, keepdims=True)
    v = ((x - m) ** 2).mean(-1, keepdims=True)
    return (x - m) / jnp.sqrt(v + eps) * w + b


def _forward_shard(x, p):
    """x: [BL, T, N, 4] local batch shard; p: dict of replicated params."""
    Bl = BL
    static_adj = p['static_adj']
    flow = x[..., :1]
    hour = x[..., 1:2]; weekend = x[..., 2:3]; day = x[..., 3:4]
    node_emb = jnp.broadcast_to(p['node_embedding'][None, None], (Bl, T, N, E))
    hour_emb = hour @ p['hour_w'].T + p['hour_b']
    day_emb = day @ p['day_w'].T + p['day_b']
    weekend_emb = weekend @ p['weekend_w'].T + p['weekend_b']
    h = jnp.concatenate([flow, node_emb, hour_emb, day_emb, weekend_emb], axis=-1)
    h = h.transpose(0, 3, 2, 1)  # [Bl,F,N,T]
    h = jnp.einsum('of,bfnt->bont', p['in_w'], h) + p['in_b'][None, :, None, None]
    pos = jnp.arange(100, dtype=jnp.float32)[:, None]
    div = jnp.exp(jnp.arange(0, C, 2, dtype=jnp.float32) * (-math.log(10000.0) / C))
    pe = jnp.stack([jnp.sin(pos * div), jnp.cos(pos * div)], axis=-1).reshape(100, C)
    h = h + pe[:T].T[None, :, None, :]
    adj_norm = static_adj / jnp.maximum(jnp.abs(static_adj).sum(-1, keepdims=True), 1e-12)
    zero_mask = (adj_norm == 0)[None]
    attn_mats = []
    sqrtC = math.sqrt(C); sqrtHD = math.sqrt(HD)
    for l in range(L):
        hp = h.transpose(0, 1, 3, 2)  # [Bl,C,T,N]
        q = jnp.einsum('oc,bctn->botn', p['sa_qw'][l], hp) + p['sa_qb'][l][None, :, None, None]
        k = jnp.einsum('oc,bctn->botn', p['sa_kw'][l], hp) + p['sa_kb'][l][None, :, None, None]
        v = jnp.einsum('oc,bctn->botn', p['sa_vw'][l], hp) + p['sa_vb'][l][None, :, None, None]
        C8 = q.shape[1]
        q2 = q.reshape(Bl * T, C8, N).transpose(0, 2, 1)
        k2 = k.reshape(Bl * T, C8, N)
        v2 = v.reshape(Bl * T, C, N)
        attn = jax.nn.softmax(jnp.einsum('bnc,bcm->bnm', q2, k2) / sqrtC, axis=-1)
        attn = jnp.where(zero_mask, -jnp.float32(np.inf), attn)
        attn = jax.nn.softmax(attn, axis=-1)
        sp = jnp.einsum('bcn,bmn->bcm', v2, attn).reshape(Bl, T, C, N).transpose(0, 2, 3, 1)
        attn_mats.append(attn.reshape(Bl, T, N, N))
        h = h + sp
        h = _ln(h.transpose(0, 3, 2, 1), p['ln1_w'][l], p['ln1_b'][l]).transpose(0, 3, 2, 1)
        xt = h.transpose(0, 2, 3, 1)  # [Bl,N,T,C]

        def heads(w, b):
            return (xt @ w.T + b).reshape(Bl, N, T, NH, HD).transpose(0, 1, 3, 2, 4)
        tq = heads(p['ta_qw'][l], p['ta_qb'][l])
        tk = heads(p['ta_kw'][l], p['ta_kb'][l])
        tv = heads(p['ta_vw'][l], p['ta_vb'][l])
        scores = jnp.einsum('bnhtd,bnhsd->bnhts', tq, tk) / sqrtHD
        ta = jnp.einsum('bnhts,bnhsd->bnhtd', jax.nn.softmax(scores, axis=-1), tv)
        ta = ta.transpose(0, 1, 3, 2, 4).reshape(Bl, N, T, C) @ p['ta_ow'][l].T + p['ta_ob'][l]
        h = h + ta.transpose(0, 3, 1, 2)
        hl = _ln(h.transpose(0, 3, 2, 1), p['ln2_w'][l], p['ln2_b'][l])
        ffn = jax.nn.relu(hl @ p['ffn_w1'][l].T + p['ffn_b1'][l]) @ p['ffn_w2'][l].T + p['ffn_b2'][l]
        h = (hl + ffn).transpose(0, 3, 2, 1)
    seq = h.transpose(0, 2, 3, 1).reshape(Bl * N, T, C).transpose(1, 0, 2)
    for l in range(2):
        xp = jnp.einsum('tbc,gc->tbg', seq, p['gru_wih'][l]) + p['gru_bih'][l]
        whh = p['gru_whh'][l]; bhh = p['gru_bhh'][l]

        def step(hprev, xg):
            hg = hprev @ whh.T + bhh
            r = jax.nn.sigmoid(xg[:, :C] + hg[:, :C])
            z = jax.nn.sigmoid(xg[:, C:2 * C] + hg[:, C:2 * C])
            n = jnp.tanh(xg[:, 2 * C:] + r * hg[:, 2 * C:])
            hnew = (1 - z) * n + z * hprev
            return hnew, hnew
        _, seq = jax.lax.scan(step, jnp.zeros((Bl * N, C), dtype=seq.dtype), xp)
    h = seq.transpose(1, 0, 2).reshape(Bl, N, T, C).transpose(0, 3, 1, 2)
    flat = h.transpose(0, 2, 1, 3).reshape(Bl * N, C * T)
    out = (flat @ p['pred_w'].T + p['pred_b']).reshape(Bl, N, HOR).transpose(0, 2, 1)
    attn_stack = jnp.stack(attn_mats)  # [L, Bl, T, N, N]
    return out, attn_stack


_COMPILED = None


def _get_compiled():
    global _COMPILED
    if _COMPILED is None:
        devs = jax.devices()[:NDEV]
        mesh = Mesh(np.array(devs), ('x',))
        fn = shard_map(
            _forward_shard, mesh=mesh,
            in_specs=(P('x'), P()),
            out_specs=(P('x'), P(None, 'x')),
            check_rep=False,
        )
        _COMPILED = jax.jit(fn)
    return _COMPILED


def kernel(**inputs):
    x = np.asarray(inputs['x'], dtype=np.float32)
    params = {k: np.asarray(inputs[k], dtype=np.float32) for k in PARAM_NAMES}
    fn = _get_compiled()
    out, attn = fn(x, params)
    out = np.asarray(jax.device_get(out), dtype=np.float32)
    attn = np.asarray(jax.device_get(attn), dtype=np.float32)
    return out, attn
